# revision 5
# baseline (speedup 1.0000x reference)
"""Self-contained Trainium2 Bass kernel for the DecoConv GNN layer.

kernel(**inputs) takes the full (unsharded) numpy inputs and returns the full
[100000, 64] fp32 output. Internally: shards destination nodes across the 8
NeuronCores, builds + compiles one SPMD Bass/Tile program on first call, and
runs it via concourse's PJRT path on cores 0-7.
"""
import sys
if '/opt/trn_rl_repo' not in sys.path:
    sys.path.insert(0, '/opt/trn_rl_repo')

import numpy as np

# ======================================================================
# environment fixups (walrus single-sync-wait limit, NTFF hook, uploads)
# ======================================================================
"""Split multi-wait instructions in BIR JSON: this container's walrus supports
only ONE sync wait per instruction. Extra waits are moved onto standalone
EventSemaphore instructions inserted immediately before (same engine, in-order)."""
import orjson

# opcodes that must stay glued to the following instruction (weights load + matmul)
_GLUE_PREV = {"TensorLoad", "LoadStationary", "TensorLoadWeights", "LdWeights"}

def split_multiwaits_json(bir_bytes: bytes) -> bytes:
    d = orjson.loads(bir_bytes)
    n_split = 0
    uid = [0]
    for fn in d.get("functions", []):
        for blk in fn.get("blocks", []):
            insts = blk.get("instructions", [])
            out = []
            for inst in insts:
                si = inst.get("sync_info") or {}
                waits = si.get("on_wait") or []
                if len(waits) > 1:
                    n_split += 1
                    pre = []
                    for w in waits:
                        uid[0] += 1
                        pre.append({
                            "debug": inst.get("debug", 0),
                            "engine": inst["engine"],
                            "ins": [],
                            "name": f"{inst['name']}_sw{uid[0]}",
                            "opcode": "EventSemaphore",
                            "outs": [],
                            "sync_info": {"on_update": [], "on_wait": [w]},
                        })
                    si["on_wait"] = []
                    inst["sync_info"] = si
                    # insert before a glued weights-load if present
                    ip = len(out)
                    while ip > 0 and out[ip-1].get("opcode") in _GLUE_PREV and out[ip-1].get("engine") == inst["engine"]:
                        ip -= 1
                    out[ip:ip] = pre
                out.append(inst)
            blk["instructions"] = out
    return orjson.dumps(d), n_split

_installed = False

def _make_ntff_hook(so_path="/opt/axon/libaxon_pjrt.so"):
    import contextlib, ctypes
    lib = ctypes.CDLL(so_path)
    if not hasattr(lib, "axon_start_nrt_profile"):
        return None
    lib.axon_start_nrt_profile.argtypes = [ctypes.POINTER(ctypes.c_int64), ctypes.c_size_t]
    lib.axon_start_nrt_profile.restype = ctypes.c_int64
    lib.axon_stop_nrt_profile.argtypes = [ctypes.c_char_p]
    lib.axon_stop_nrt_profile.restype = ctypes.c_int64

    @contextlib.contextmanager
    def _hook(output_dir, device_ids):
        import jax
        jax.devices()
        if device_ids:
            ids = (ctypes.c_int64 * len(device_ids))(*device_ids)
            rc = lib.axon_start_nrt_profile(ids, len(device_ids))
        else:
            rc = lib.axon_start_nrt_profile(None, 0)
        if rc != 0:
            raise RuntimeError(f"axon_start_nrt_profile rc={rc}")
        try:
            yield
        finally:
            n = lib.axon_stop_nrt_profile(str(output_dir).encode())
            if n < 0:
                raise RuntimeError(f"axon_stop_nrt_profile rc={n}")
    return _hook


def install():
    global _installed
    if _installed:
        return
    from concourse import bass2jax, bass_utils
    orig = bass_utils.compile_bir_kernel
    def patched(ant_bir_str, compile_dir_path, neff_name, **kw):
        fixed, n = split_multiwaits_json(ant_bir_str if isinstance(ant_bir_str, bytes) else ant_bir_str.encode())
        return orig(fixed, compile_dir_path, neff_name=neff_name, **kw)
    bass2jax.compile_bir_kernel = patched

    # antenv.axon_hooks shim so run_bass_kernel_spmd(trace=True) works
    import sys, types
    try:
        import antenv.axon_hooks  # noqa
    except ImportError:
        hook = _make_ntff_hook()
        mod = types.ModuleType("antenv.axon_hooks")
        mod.get_axon_ntff_profile_hook = lambda: hook
        mod.set_axon_ntff_profile_hook = lambda h: None
        sys.modules["antenv.axon_hooks"] = mod
        import antenv
        antenv.axon_hooks = mod

    # no-op the artifact upload (no bucket access in this sandbox)
    bass_utils.upload_artifacts = lambda tmpdir: f"local:{tmpdir}"
    _installed = True


# ======================================================================
# kernel build + host pre/post processing
# ======================================================================
"""GNN message-passing kernel for TRN2 (dest-sharded SpMM + Linear + residual + BN + ReLU).

Layout strategy (per core):
- Destination nodes sharded: core c owns rows [c*S, (c+1)*S).
- Host groups the core's edges by (128-dest tile, source bucket). Tiles are
  ranked by descending edge count (so the SPMD-shared per-position block
  counts, the max across cores, track each core's actual counts), then
  dealt serpentine-fashion into NCHUNK chunks of near-equal block totals.
- Device loop is chunk-major: per chunk ONE dma_gather per source bucket
  (few big gathers instead of one per tile*bucket: SWDGE descriptor
  generation costs ~1us fixed per gather instruction on gpsimd).
- x is stored in HBM as [N, 128] bf16 rows (features in [:64], zero pad to a
  256-byte row, dma_gather's minimum element size).
- Per dest tile, VectorE builds the val-scaled one-hot scatter matrix in one
  tensor_scalar op per 128-edge block: mm[e, i] = (iota[i] == r_e) * val_e,
  with r/val as per-partition fp32 scalars. Both tensor operands are packed
  bf16 SBUF, so this runs in the DVE 4x perf mode (the old tensor_tensor
  broadcast build ran at 1x). TensorE accumulates h1^T = sum_k Xg_k^T @ M_k
  per dest tile in PSUM; features then live on partitions, so Linear,
  residual add, BN stats (free-dim reductions + AllReduce) and the fused
  scale/shift/ReLU are cheap batched ops.
- The Linear bias is dropped entirely: training-mode BatchNorm output is
  mathematically invariant to a constant per-feature shift.
"""

import numpy as np
import ml_dtypes

BF16 = ml_dtypes.bfloat16
D = 64
TILE = 128
XROW = 128          # padded bf16 row length of x in HBM (256 bytes)
BK = 25000          # source-bucket rows (int16 index range)
NCHUNK = 12         # gather chunks (serpentine-balanced tile sets)
GMAX = 8            # max blocks (1024 idxs) per dma_gather instruction (HW limit)
GROUP_TILES = 4     # tiles per linear/residual group (512 psum columns)


# ---------------------------------------------------------------- host prep

def host_prep(x, adj_val, adj_row, adj_col, W, b, n_cores):
    N = x.shape[0]
    S = N // n_cores
    assert S * n_cores == N
    n_tiles = (S + TILE - 1) // TILE
    S_pad = n_tiles * TILE
    nbuck = (N + BK - 1) // BK

    adj_row = np.asarray(adj_row)
    adj_col = np.asarray(adj_col)
    adj_val = np.asarray(adj_val)

    # serpentine deal of tile ranks into chunks (balances blocks per chunk)
    pattern = []
    while len(pattern) < n_tiles:
        pattern.extend(range(NCHUNK))
        pattern.extend(reversed(range(NCHUNK)))
    pattern = pattern[:n_tiles]
    rank_chunks = [[] for _ in range(NCHUNK)]
    for r, c in enumerate(pattern):
        rank_chunks[c].append(r)
    rankseq = [r for ch in rank_chunks for r in ch]        # jpos -> rank
    chunk_sizes = [len(ch) for ch in rank_chunks]
    chunk_jpos = []
    j0 = 0
    for cs in chunk_sizes:
        chunk_jpos.append(list(range(j0, j0 + cs)))
        j0 += cs

    core_of_edge = adj_row // S

    edges_by_core = []
    cnt_pos = np.zeros((n_cores, n_tiles, nbuck), dtype=np.int64)
    orders = []
    for c in range(n_cores):
        m = core_of_edge == c
        er = adj_row[m] - c * S
        ec = adj_col[m]
        ev = adj_val[m]
        t = er // TILE
        q = ec // BK
        cnt = np.zeros((n_tiles, nbuck), dtype=np.int64)
        np.add.at(cnt, (t, q), 1)
        order = np.argsort(-cnt.sum(1), kind="stable")     # rank -> tile id
        orders.append(order)
        # tile at jpos p is order[rankseq[p]]
        for p in range(n_tiles):
            cnt_pos[c, p] = cnt[order[rankseq[p]]]
        edges_by_core.append((er, ec, ev, t, q))

    nb_sh = (np.ceil(cnt_pos.max(0) / TILE)).astype(np.int64)  # [jpos, q]
    empty = nb_sh.sum(1) == 0
    nb_sh[empty, 0] = 1

    # global block layout: chunk-major, bucket-major within chunk,
    # jpos-major within (chunk, bucket) segment
    blk_base = np.zeros((n_tiles, nbuck), dtype=np.int64)
    chunks = []
    gblk = 0
    for ch in range(NCHUNK):
        cbase = gblk
        segs = []
        for q in range(nbuck):
            seg_start = gblk - cbase
            for p in chunk_jpos[ch]:
                blk_base[p, q] = gblk
                gblk += nb_sh[p, q]
            segs.append((seg_start, int(gblk - cbase - seg_start)))
        tiles = []
        for p in chunk_jpos[ch]:
            blks = []
            for q in range(nbuck):
                for k in range(nb_sh[p, q]):
                    blks.append(int(blk_base[p, q] + k))
            tiles.append((p, blks))
        chunks.append(dict(cbase=int(cbase), cblocks=int(gblk - cbase),
                           segs=segs, tiles=tiles))
    B = int(gblk)

    # linear/residual groups: consecutive jpos runs of <=GROUP_TILES within a chunk
    groups = []
    for ch in range(NCHUNK):
        js = chunk_jpos[ch]
        for i in range(0, len(js), GROUP_TILES):
            sub = js[i:i + GROUP_TILES]
            groups.append((sub[0] * TILE, len(sub) * TILE, ch))
    n_groups = len(groups)

    per_core = []
    for c in range(n_cores):
        er, ec, ev, t, q = edges_by_core[c]
        order = orders[c]
        jpos_of_tile = np.empty(n_tiles, dtype=np.int64)
        for p in range(n_tiles):
            jpos_of_tile[order[rankseq[p]]] = p
        j = jpos_of_tile[t]
        gid = j * nbuck + q                                # group id (jpos, q)

        val32 = np.zeros((TILE, B), dtype=np.float32)
        r32 = np.zeros((TILE, B), dtype=np.float32)
        idx16 = np.zeros((TILE, B * 8), dtype=np.int16)

        sidx = np.argsort(gid, kind="stable")
        gg = gid[sidx]
        cnt_g = np.bincount(gg, minlength=n_tiles * nbuck)
        start = np.zeros(n_tiles * nbuck, dtype=np.int64)
        start[1:] = np.cumsum(cnt_g)[:-1]
        pos = np.arange(len(gg)) - start[gg]
        blk = blk_base.reshape(-1)[gg] + pos // TILE
        sp = pos % TILE
        val32[sp, blk] = ev[sidx].astype(np.float32)
        r32[sp, blk] = (er[sidx] - t[sidx] * TILE).astype(np.float32)
        rel = (ec[sidx] - q[sidx] * BK).astype(np.int16)
        colpos = blk * 8 + sp // 16
        rowpos = sp % 16
        for g in range(8):
            idx16[rowpos + 16 * g, colpos] = rel

        # transposed residual input, jpos-order permuted, bf16
        xres = np.zeros((D, S_pad), dtype=BF16)
        xc = x[c * S : (c + 1) * S]
        tile_of_jpos = [int(order[rankseq[p]]) for p in range(n_tiles)]
        for p, tt in enumerate(tile_of_jpos):
            lo = tt * TILE
            hi = min(lo + TILE, S)
            nvalid = hi - lo
            xres[:, p * TILE : p * TILE + nvalid] = xc[lo:hi].T.astype(BF16)

        per_core.append(
            dict(val32=val32, r32=r32, idx16=idx16, xres=xres,
                 tile_of_jpos=tile_of_jpos)
        )

    waug = np.asarray(W, dtype=np.float32).T.astype(BF16)  # [in, out]; bias dropped (BN-invariant)
    x_pad = np.zeros((N, XROW), dtype=BF16)
    x_pad[:, :D] = np.asarray(x).astype(BF16)

    meta = dict(N=N, S=S, n_tiles=n_tiles, S_pad=S_pad, B=B, nbuck=nbuck,
                nb_sh=nb_sh.tolist(), chunks=chunks, groups=groups,
                n_groups=n_groups)
    return meta, per_core, waug, x_pad


def host_post(results, metas, n_cores):
    """Assemble full [N, 64] fp32 output from per-core transposed outputs."""
    meta = metas["meta"]
    S = meta["S"]
    N = meta["N"]
    out = np.empty((N, D), dtype=np.float32)
    for c in range(n_cores):
        dev = results[c]  # [64, S_pad]
        tile_of_jpos = metas["per_core"][c]["tile_of_jpos"]
        for p, tt in enumerate(tile_of_jpos):
            lo = tt * TILE
            hi = min(lo + TILE, S)
            nvalid = hi - lo
            out[c * S + lo : c * S + hi] = dev[:, p * TILE : p * TILE + nvalid].T
    return out


# ---------------------------------------------------------------- device build

def build_nc(meta, n_cores, eps, replica_groups=None):
    from concourse import bass, mybir, tile

    N = meta["N"]
    S_pad = meta["S_pad"]
    n_tiles = meta["n_tiles"]
    B = meta["B"]
    nbuck = meta["nbuck"]
    chunks = meta["chunks"]
    groups = meta["groups"]
    n_groups = meta["n_groups"]
    ch_max = max(ch["cblocks"] for ch in chunks)
    nbt_max = max(len(blks) for ch in chunks for (_, blks) in ch["tiles"])
    f32 = mybir.dt.float32
    bf16 = mybir.dt.bfloat16
    i16 = mybir.dt.int16
    i32 = mybir.dt.int32

    nc = bass.Bass(debug=False, num_swdge_queues=4)
    x_d = nc.declare_dram_parameter("x_pad", [N, XROW], bf16, isOutput=False)
    idx_d = nc.declare_dram_parameter("idx16", [TILE, B * 8], i16, isOutput=False)
    val_d = nc.declare_dram_parameter("val32", [TILE, B], f32, isOutput=False)
    r_d = nc.declare_dram_parameter("r32", [TILE, B], f32, isOutput=False)
    xres_d = nc.declare_dram_parameter("xres", [D, S_pad], bf16, isOutput=False)
    waug_d = nc.declare_dram_parameter("waug", [D, D], bf16, isOutput=False)
    gam_d = nc.declare_dram_parameter("gam", [D, 1], f32, isOutput=False)
    bet_d = nc.declare_dram_parameter("bet", [D, 1], f32, isOutput=False)
    out_d = nc.declare_dram_parameter("outp", [D, S_pad], f32, isOutput=True)

    if replica_groups is not None:
        cc_in = nc.dram_tensor("cc_in", [D, 2], f32)
        cc_out = nc.dram_tensor("cc_out", [D, 2], f32, addr_space="Shared")

    with tile.TileContext(nc) as tc:
        with (
            tc.tile_pool(name="const", bufs=1) as constp,
            tc.tile_pool(name="big", bufs=1) as bigp,
            tc.tile_pool(name="xg", bufs=2) as xgp,
            tc.tile_pool(name="idx", bufs=2) as idxp,
            tc.tile_pool(name="mm", bufs=3) as mmp,
            tc.tile_pool(name="h1", bufs=2) as h1p,
            tc.tile_pool(name="xres", bufs=2) as xrp,
            tc.tile_pool(name="og", bufs=2) as ogp,
            tc.tile_pool(name="sq", bufs=2) as sqp,
            tc.tile_pool(name="psA", bufs=4, space="PSUM") as psA,
            tc.tile_pool(name="psB", bufs=2, space="PSUM") as psB,
        ):
            val_sb = bigp.tile([TILE, B], f32)
            r_sb = bigp.tile([TILE, B], f32)
            h3_sb = bigp.tile([D, S_pad], f32)
            waug_sb = constp.tile([D, D], bf16)
            gam_sb = constp.tile([D, 1], f32)
            bet_sb = constp.tile([D, 1], f32)
            iota_i = constp.tile([TILE, TILE], i32)
            iota_b = constp.tile([TILE, TILE], bf16)
            stat_s = constp.tile([D, n_groups], f32)
            stat_q = constp.tile([D, n_groups], f32)

            nc.sync.dma_start(val_sb[:], val_d[:])
            nc.sync.dma_start(r_sb[:], r_d[:])
            nc.sync.dma_start(waug_sb[:], waug_d[:])
            nc.sync.dma_start(gam_sb[:], gam_d[:])
            nc.sync.dma_start(bet_sb[:], bet_d[:])

            nc.gpsimd.iota(iota_i[:], pattern=[[1, TILE]], base=0,
                           channel_multiplier=0)
            nc.vector.tensor_copy(iota_b[:], iota_i[:])

            # one register per distinct gather size (to_reg doesn't cache)
            nidx_regs = {}
            for ch in chunks:
                for (_, nbq) in ch["segs"]:
                    for sb0 in range(0, nbq, GMAX):
                        nbg = min(GMAX, nbq - sb0)
                        if nbg * TILE not in nidx_regs:
                            nidx_regs[nbg * TILE] = nc.gpsimd.to_reg(nbg * TILE)

            gi = 0
            qrot = 0
            for ci, ch in enumerate(chunks):
                cbase = ch["cbase"]
                cblocks = ch["cblocks"]
                idx_sb = idxp.tile([TILE, ch_max * 8], i16, tag="idx")
                nc.sync.dma_start(idx_sb[:, : cblocks * 8],
                                  idx_d[:, cbase * 8 : (cbase + cblocks) * 8])
                xg = xgp.tile([TILE, ch_max * XROW], bf16, tag="xg")
                for q in range(nbuck):
                    seg_start, nbq = ch["segs"][q]
                    if nbq == 0:
                        continue
                    nrow = min(BK, N - q * BK)
                    # HW caps one dma_gather at 1024 idxs (8 blocks): the Q7
                    # idx-read stream / ring window rejects larger gathers.
                    for sb0 in range(0, nbq, GMAX):
                        nbg = min(GMAX, nbq - sb0)
                        s0 = seg_start + sb0
                        nc.gpsimd.dma_gather(
                            out_ap=xg[:, s0 * XROW : (s0 + nbg) * XROW]
                                .rearrange("p (b e) -> p b e", e=XROW),
                            in_ap=x_d[q * BK : q * BK + nrow, :],
                            idxs_ap=idx_sb[:, s0 * 8 : (s0 + nbg) * 8],
                            num_idxs=nbg * TILE,
                            num_idxs_reg=nidx_regs[nbg * TILE],
                            elem_size=XROW,
                            elem_step=XROW,
                            queue_num=qrot % 4,
                        )
                        qrot += 1

                h1g = None
                gcols = 0
                for (jpos, blks) in ch["tiles"]:
                    nbt = len(blks)
                    if h1g is None:
                        goff, gncols, _ = groups[gi]
                        h1g = h1p.tile([D, GROUP_TILES * TILE], bf16, tag="h1")
                        gcols = 0
                    mm = mmp.tile([TILE, nbt_max * TILE], bf16, tag="mm")
                    for k, g in enumerate(blks):
                        nc.vector.tensor_scalar(
                            out=mm[:, k * TILE : (k + 1) * TILE],
                            in0=iota_b[:],
                            scalar1=r_sb[:, g : g + 1],
                            scalar2=val_sb[:, g : g + 1],
                            op0=bass.mybir.AluOpType.is_equal,
                            op1=bass.mybir.AluOpType.mult,
                        )
                    ps = psA.tile([D, TILE], f32, tag="ps")
                    for k, g in enumerate(blks):
                        lo = g - cbase
                        nc.tensor.matmul(
                            ps[:],
                            lhsT=xg[:, lo * XROW : lo * XROW + D],
                            rhs=mm[:, k * TILE : (k + 1) * TILE],
                            start=(k == 0),
                            stop=(k == nbt - 1),
                        )
                    nc.scalar.copy(h1g[:, gcols : gcols + TILE], ps[:])
                    gcols += TILE

                    if gcols == gncols:
                        # finish group gi: Linear, residual, stats
                        off = goff
                        ncols = gncols
                        xres_t = xrp.tile([D, GROUP_TILES * TILE], bf16, tag="xr")
                        nc.sync.dma_start(xres_t[:, :ncols],
                                          xres_d[:, off : off + ncols])
                        ps2 = psB.tile([D, GROUP_TILES * TILE], f32, tag="ps2")
                        nc.tensor.matmul(
                            ps2[:, :ncols],
                            lhsT=waug_sb[:],
                            rhs=h1g[:, :ncols],
                            start=True, stop=True,
                        )
                        nc.vector.tensor_tensor(
                            out=h3_sb[:, off : off + ncols],
                            in0=ps2[:, :ncols],
                            in1=xres_t[:, :ncols],
                            op=bass.mybir.AluOpType.add,
                        )
                        sq_scr = sqp.tile([D, GROUP_TILES * TILE], f32, tag="sq")
                        nc.scalar.activation(
                            sq_scr[:, :ncols],
                            h3_sb[:, off : off + ncols],
                            bass.mybir.ActivationFunctionType.Square,
                            accum_out=stat_q[:, gi : gi + 1],
                        )
                        nc.vector.reduce_sum(
                            stat_s[:, gi : gi + 1],
                            h3_sb[:, off : off + ncols],
                            axis=bass.mybir.AxisListType.X,
                        )
                        gi += 1
                        h1g = None
                assert h1g is None, "chunk tiles not group-aligned"

            stats2 = constp.tile([D, 2], f32)
            nc.vector.reduce_sum(stats2[:, 0:1], stat_s[:],
                                 axis=bass.mybir.AxisListType.X)
            nc.vector.reduce_sum(stats2[:, 1:2], stat_q[:],
                                 axis=bass.mybir.AxisListType.X)

            statsg = constp.tile([D, 2], f32)
            if replica_groups is not None:
                nc.gpsimd.dma_start(cc_in[:], stats2[:])
                nc.gpsimd.collective_compute(
                    "AllReduce",
                    bass.mybir.AluOpType.add,
                    replica_groups=replica_groups,
                    ins=[cc_in[:]],
                    outs=[cc_out[:]],
                )
                nc.gpsimd.dma_start(statsg[:], cc_out[:])
            else:
                nc.vector.tensor_copy(statsg[:], stats2[:])

            # finalize BN constants: A = gamma / sqrt(var + eps), Bc = beta - mean*A
            eps_sb = constp.tile([D, 1], f32)
            nc.gpsimd.memset(eps_sb[:], float(eps))
            mean = constp.tile([D, 1], f32)
            esq = constp.tile([D, 1], f32)
            var = constp.tile([D, 1], f32)
            sd = constp.tile([D, 1], f32)
            rsd = constp.tile([D, 1], f32)
            A = constp.tile([D, 1], f32)
            Bc = constp.tile([D, 1], f32)
            inv_n = 1.0 / float(N)
            nc.vector.tensor_scalar_mul(mean[:], statsg[:, 0:1], inv_n)
            nc.vector.tensor_scalar_mul(esq[:], statsg[:, 1:2], inv_n)
            nc.vector.tensor_tensor(out=var[:], in0=mean[:], in1=mean[:],
                                    op=bass.mybir.AluOpType.mult)
            nc.vector.tensor_tensor(out=var[:], in0=esq[:], in1=var[:],
                                    op=bass.mybir.AluOpType.subtract)
            nc.scalar.activation(sd[:], var[:],
                                 bass.mybir.ActivationFunctionType.Sqrt,
                                 bias=eps_sb[:, 0:1], scale=1.0)
            nc.vector.reciprocal(rsd[:], sd[:])
            nc.vector.tensor_tensor(out=A[:], in0=rsd[:], in1=gam_sb[:],
                                    op=bass.mybir.AluOpType.mult)
            nc.vector.tensor_tensor(out=Bc[:], in0=mean[:], in1=A[:],
                                    op=bass.mybir.AluOpType.mult)
            nc.vector.tensor_tensor(out=Bc[:], in0=bet_sb[:], in1=Bc[:],
                                    op=bass.mybir.AluOpType.subtract)

            # apply BN + ReLU per group, stream out
            for g in range(n_groups):
                off, ncols, _ = groups[g]
                out_g = ogp.tile([D, GROUP_TILES * TILE], f32, tag="og")
                nc.scalar.activation(out_g[:, :ncols],
                                     h3_sb[:, off : off + ncols],
                                     bass.mybir.ActivationFunctionType.Relu,
                                     bias=Bc[:, 0:1], scale=A[:, 0:1])
                nc.sync.dma_start(out_d[:, off : off + ncols],
                                  out_g[:, :ncols])

    # Raw Bass (Tile) skips Bacc's library/ISA lowering passes; without them
    # the extended instructions (DMAGatherAnt) have empty .instr bytes and
    # walrus fails with "ISA wrong length", and no LOAD_LIB is emitted.
    import bass_rust as _bass_rust
    from concourse.library_config import all_libraries, standard
    inst_type_to_lib_mask = {}
    for lib in all_libraries:
        for inst_type in lib.instructions:
            inst_type_to_lib_mask[inst_type] = inst_type_to_lib_mask.get(
                inst_type, 0) | (1 << lib.index)
    _bass_rust.insert_library_loads(
        nc, inst_type_to_lib_mask, len(all_libraries), standard.index)
    mybir.codegen_inst_isa_subclasses(nc)
    return nc


def make_in_maps(meta, per_core, waug, x_pad, gamma, beta, n_cores):
    maps = []
    for c in range(n_cores):
        pc = per_core[c]
        maps.append({
            "x_pad": x_pad,
            "idx16": pc["idx16"],
            "val32": pc["val32"],
            "r32": pc["r32"],
            "xres": pc["xres"],
            "waug": waug,
            "gam": np.asarray(gamma, dtype=np.float32).reshape(D, 1),
            "bet": np.asarray(beta, dtype=np.float32).reshape(D, 1),
        })
    return maps


# ======================================================================
# entry point
# ======================================================================
_CACHE = {}

EPS = 1e-5
N_CORES = 8


def kernel(x, adj_val, W, b, gamma, beta, adj_row, adj_col):
    install()
    x = np.asarray(x); adj_val = np.asarray(adj_val)
    W = np.asarray(W); b = np.asarray(b)
    gamma = np.asarray(gamma); beta = np.asarray(beta)
    adj_row = np.asarray(adj_row).astype(np.int64)
    adj_col = np.asarray(adj_col).astype(np.int64)

    meta, per_core, waug, x_pad = host_prep(
        x, adj_val, adj_row, adj_col, W, b, N_CORES)
    in_maps = make_in_maps(meta, per_core, waug, x_pad, gamma, beta, N_CORES)

    key = (meta["B"], tuple(tuple(v) for v in meta["nb_sh"]))
    if key not in _CACHE:
        nc = build_nc(meta, N_CORES, EPS,
                      replica_groups=[list(range(N_CORES))])
        _CACHE[key] = nc
    nc = _CACHE[key]

    from concourse.bass_utils import run_bass_kernel_spmd
    res = run_bass_kernel_spmd(nc, in_maps, list(range(N_CORES)))
    out = host_post([res.results[c]["outp"] for c in range(N_CORES)],
                    dict(meta=meta, per_core=per_core), N_CORES)
    return out.astype(np.float32)
